# revision 6
# baseline (speedup 1.0000x reference)
"""GCN encoder on 8 trn2 cores, v2.

Key changes vs v1:
  - L1 messages are host-expanded (xg = x[src] fp16, per-edge, chunk layout)
    and STREAMED via affine HWDGE DMA - no per-edge descriptor generation.
  - L2 gathers h rows via dma_gather of 4-node groups (512B descs, int16
    idx = slot>>2, no range buckets); group-slot selection is fused into a
    512-wide one-hot S4 so each 128-edge chunk costs 1 DVE build + 4 PE
    matmuls.
  - Windows are chunk-contiguous (no WG interleave), caps degree-balanced.
"""
import numpy as np
from contextlib import ExitStack

import concourse.bacc as bacc
import concourse.bass as bass
import concourse.mybir as mybir
from concourse.bass_utils import run_bass_kernel_spmd
from concourse.library_config import mlp

F32 = mybir.dt.float32
F16 = mybir.dt.float16
I16 = mybir.dt.int16
AF = mybir.ActivationFunctionType
OP = mybir.AluOpType

EPS = 1e-5
TRACE = False

CFG_FULL = dict(n_nodes=100000, n_edges=1600000, n_cores=8,
                slots_per_core=12544, in_dim=128, hid_dim=64, emb_dim=128,
                n_graphs=256)


# ================================================================ host prep
def _degree_balanced_perm(dst, n_nodes, n_windows, wsize):
    import heapq
    deg = np.bincount(dst, minlength=n_nodes)
    order = np.argsort(-deg, kind="stable")
    heap = [(0, w) for w in range(n_windows)]
    heapq.heapify(heap)
    counts = np.zeros(n_windows, np.int64)
    slot = np.empty(n_nodes, np.int64)
    degs = deg[order]
    for i in range(n_nodes):
        load, w = heapq.heappop(heap)
        slot[order[i]] = w * wsize + counts[w]
        counts[w] += 1
        if counts[w] < wsize:
            heapq.heappush(heap, (load + int(degs[i]), w))
    return slot


def _wrap16(flat):
    n = flat.size
    w = flat.reshape(n // 16, 16).T.astype(np.int16)
    return np.tile(w, (8, 1))


def _host_prep(x, edge_index, edge_weight, batch_vec, cfg):
    NC, SPC = cfg["n_cores"], cfg["slots_per_core"]
    W = 128
    NWC = SPC // W
    IN = cfg["in_dim"]
    n_nodes = cfg["n_nodes"]

    src = np.asarray(edge_index[0], np.int64)
    dst = np.asarray(edge_index[1], np.int64)
    ew = np.asarray(edge_weight, np.float32)

    slot = _degree_balanced_perm(dst, n_nodes, NC * NWC, W)

    sslot, dslot = slot[src], slot[dst]
    core = dslot // SPC
    wloc = (dslot % SPC) // W
    dstoff = (dslot % W).astype(np.float32)
    grp = (sslot >> 2).astype(np.int64)
    comb = (dstoff + 128.0 * (sslot & 3)).astype(np.float32)

    # caps per window = max over cores, rounded to 128
    key = core * NWC + wloc
    cnt = np.bincount(key, minlength=NC * NWC).reshape(NC, NWC)
    caps = np.maximum(128, ((cnt.max(axis=0) + 127) // 128) * 128)  # [NWC]

    nch_w = caps // 128
    wchunk0 = np.concatenate([[0], np.cumsum(nch_w)])
    n_chunks = int(wchunk0[-1])
    chunk_window = np.repeat(np.arange(NWC), nch_w)
    wfirst = wchunk0[:-1]
    wlast = wchunk0[1:] - 1

    calls = []
    k = 0
    while k < n_chunks:
        n = min(8, n_chunks - k)
        calls.append((k, n))
        k += n

    x16 = np.asarray(x, np.float16)
    idx_cores, comb_cores, ew_cores, do_cores, xg_cores = [], [], [], [], []
    for c in range(NC):
        m = core == c
        sg, cb, dv, wv, wgt, sr = (grp[m], comb[m], dstoff[m], wloc[m],
                                   ew[m], src[m])
        e_idx = np.zeros(n_chunks * 128, np.int64)
        e_cb = np.zeros(n_chunks * 128, np.float32)
        e_do = np.zeros(n_chunks * 128, np.float32)
        e_ew = np.zeros(n_chunks * 128, np.float32)
        e_src = np.zeros(n_chunks * 128, np.int64)
        for w in range(NWC):
            sel = wv == w
            n = int(sel.sum())
            s = int(wfirst[w]) * 128
            e_idx[s:s + n] = sg[sel]
            e_cb[s:s + n] = cb[sel]
            e_do[s:s + n] = dv[sel]
            e_ew[s:s + n] = wgt[sel]
            e_src[s:s + n] = sr[sel]
        idx_cores.append(_wrap16(e_idx))
        comb_cores.append(np.ascontiguousarray(
            e_cb.reshape(n_chunks, 128).T))
        do_cores.append(np.ascontiguousarray(
            e_do.reshape(n_chunks, 128).T))
        ew_cores.append(np.ascontiguousarray(
            e_ew.reshape(n_chunks, 128).T))
        # xg[p, t, :] = x[src of edge at chunk t partition p]  (ew=0 rows pad)
        xg = x16[e_src].reshape(n_chunks, 128, IN).transpose(1, 0, 2)
        xg_cores.append(np.ascontiguousarray(xg))

    gid = np.full(NC * SPC, -1.0, np.float32)
    gid[slot] = np.asarray(batch_vec, np.float32)
    msk = np.zeros(NC * SPC, np.float32)
    msk[slot] = 1.0
    gid_cores = [np.ascontiguousarray(
        gid[c * SPC:(c + 1) * SPC].reshape(NWC, W).T) for c in range(NC)]
    msk_cores = [np.ascontiguousarray(
        msk[c * SPC:(c + 1) * SPC].reshape(NWC, W).T) for c in range(NC)]

    layout = dict(n_chunks=n_chunks, calls=calls, NWC=NWC,
                  chunk_window=chunk_window.tolist(),
                  wfirst=wfirst.tolist(), wlast=wlast.tolist())
    percore = dict(idx=idx_cores, comb=comb_cores, dstoff=do_cores,
                   ew=ew_cores, xg=xg_cores, gid=gid_cores, msk=msk_cores)
    return layout, percore, slot


# ============================================================= bass program
def _build(cfg, layout):
    NC, SPC = cfg["n_cores"], cfg["slots_per_core"]
    IN, HID, EMB = cfg["in_dim"], cfg["hid_dim"], cfg["emb_dim"]
    NG = cfg["n_graphs"]
    NSLOT = NC * SPC
    NWC = layout["NWC"]
    W = 128
    n_chunks = layout["n_chunks"]
    calls = layout["calls"]
    chunk_window = layout["chunk_window"]
    wfirst, wlast = layout["wfirst"], layout["wlast"]
    n_real = cfg["n_nodes"]
    GHALF = NG // 128
    NBUF = 3
    ncalls = len(calls)
    SLAB = 64
    n_slabs = (n_chunks + SLAB - 1) // SLAB
    NGRP = NSLOT // 4          # 4-node groups in gather table

    nc = bacc.Bacc("TRN2")

    xg_d = nc.dram_tensor("xg", [128, n_chunks, IN], F16, kind="ExternalInput")
    idx_d = nc.dram_tensor("idx", [128, n_chunks * 8], I16,
                           kind="ExternalInput")
    comb_d = nc.dram_tensor("cmb", [128, n_chunks], F32, kind="ExternalInput")
    do_d = nc.dram_tensor("dof", [128, n_chunks], F32, kind="ExternalInput")
    ew_d = nc.dram_tensor("ewt", [128, n_chunks], F32, kind="ExternalInput")
    gid_d = nc.dram_tensor("gid", [128, NWC], F32, kind="ExternalInput")
    msk_d = nc.dram_tensor("msk", [128, NWC], F32, kind="ExternalInput")
    w1_d = nc.dram_tensor("w1", [IN, HID], F16, kind="ExternalInput")
    w2_d = nc.dram_tensor("w2", [HID, EMB], F16, kind="ExternalInput")
    bn_d = nc.dram_tensor("bnp", [128, 6], F32, kind="ExternalInput")
    out_d = nc.dram_tensor("pool", [GHALF, 128, EMB], F32,
                           kind="ExternalOutput")

    ag_in = nc.dram_tensor("ag_in", [SPC // 4, 4 * HID], F16)
    ag_out = nc.dram_tensor("ag_out", [NGRP, 4 * HID], F16,
                            addr_space="Shared")
    ar1_in = nc.dram_tensor("ar1_in", [HID, 2], F32)
    ar1_out = nc.dram_tensor("ar1_out", [HID, 2], F32, addr_space="Shared")
    ar2_in = nc.dram_tensor("ar2_in", [EMB, 2], F32)
    ar2_out = nc.dram_tensor("ar2_out", [EMB, 2], F32, addr_space="Shared")
    bnrow = nc.dram_tensor("bnrow", [2, EMB], F16)

    with ExitStack() as ctx:
        sb = lambda n, s, d: ctx.enter_context(nc.sbuf_tensor(n, s, d))
        sem = lambda n: ctx.enter_context(nc.semaphore(n))

        idx_sb = sb("idx_sb", [128, n_chunks * 8], I16)
        comb_sb = sb("comb_sb", [128, n_chunks], F32)
        do_sb = sb("do_sb", [128, n_chunks], F32)
        ew_sb = sb("ew_sb", [128, n_chunks], F32)
        gid_sb = sb("gid_sb", [128, NWC], F32)
        msk_sb = sb("msk_sb", [128, NWC], F16)
        mskf_sb = sb("mskf_sb", [128, NWC], F32)
        w1_sb = sb("w1_sb", [IN, HID], F16)
        w2_sb = sb("w2_sb", [HID, EMB], F16)
        bn_sb = sb("bn_sb", [128, 6], F32)
        iota_sb = sb("iota_sb", [128, W], F16)
        iot4_sb = sb("iot4_sb", [128, 512], F16)
        iotg_sb = sb("iotg_sb", [128, NG], F16)
        iotp_sb = sb("iotp_sb", [128, 1], F32)
        id16_sb = sb("id16_sb", [128, 128], F16)

        xs = [sb(f"xs_{i}", [128, SLAB, IN], F16) for i in range(2)]
        s1sl = [sb(f"s1_{i}", [128, W], F16) for i in range(8)]
        s4sl = [sb(f"s4_{i}", [128, 512], F16) for i in range(8)]
        mb2 = [sb(f"mb2_{i}", [128, 8, 4 * HID], F16) for i in range(NBUF)]
        segx_sb = [sb(f"sgx_{i}", [128, W], F16) for i in range(2)]
        s2f_sb = [sb(f"s2f_{i}", [HID, W], F16) for i in range(2)]
        happ_sb = [sb(f"hap_{i}", [HID, W], F16) for i in range(2)]
        sq_sb = [sb(f"sq_{i}", [128, W], F32) for i in range(2)]
        sq2_sb = [sb(f"sq2_{i}", [128, W], F16) for i in range(2)]
        out1h_sb = sb("out1h_sb", [HID, NWC * W], F16)
        stats1_sb = sb("stats1_sb", [HID, 2 * NWC], F32)
        h_nm = sb("h_nm", [128, NWC, HID], F16)
        out2_sb = sb("out2_sb", [128, NWC * EMB], F16)
        stat_sb = sb("stat_sb", [128, 2], F32)
        tmp_sb = sb("tmp_sb", [128, 2], F32)
        coef_sb = sb("coef_sb", [128, 2], F32)
        coefh_sb = sb("coefh_sb", [128, 2], F16)
        coefr_sb = sb("coefr_sb", [128, 2 * EMB], F16)
        gone_sb = [sb(f"gone_{i}", [128, NG], F16) for i in range(2)]
        pout_sb = sb("pout_sb", [128, GHALF * EMB], F32)

        # psum: one full bank per in-flight seg window (ring of 4); long
        # accumulation groups (stats, pool) share b4, read only at the end.
        sgt = [ctx.enter_context(nc.psum_tensor(f"sg{i}", [128, 512], F32))
               for i in range(4)]
        segq = [t[:, 0:128] for t in sgt]
        b2 = ctx.enter_context(nc.psum_tensor("b2", [128, 512], F32))
        out1_ps = [b2[:HID, 0:W], b2[:HID, W:2 * W]]
        b5 = ctx.enter_context(nc.psum_tensor("b5", [128, 1024], F16))
        hT_ps = [b5[:, 0:HID], b5[:, HID:2 * HID]]
        b3 = ctx.enter_context(nc.psum_tensor("b3", [128, 512], F32))
        out2_ps = [b3[:, 0:EMB], b3[:, EMB:2 * EMB]]
        b4 = ctx.enter_context(nc.psum_tensor("b4", [128, 512], F32))
        pool_ps = [b4[:, i * EMB:(i + 1) * EMB] for i in range(GHALF)]
        st_ps = [b4[:, 2 * EMB:2 * EMB + 1], b4[:, 2 * EMB + 1:2 * EMB + 2]]

        io = sem("io")
        xls = sem("xls")
        sdone = sem("sdone")      # S1 builds (DVE)
        sdoneG = sem("sdoneG")    # S1 builds (GpSimd)
        s4done = sem("s4done")    # S4 builds
        pchunk = sem("pchunk")    # L1 chunk matmuls
        pchunk2 = sem("pchunk2")  # L2 chunk matmuls (1 per chunk)
        segcp = sem("segcp")      # ACT segx copies (L1)
        seg2cp = sem("seg2cp")    # ACT seg2f copies (L2)
        w1d = sem("w1d")
        w2d = sem("w2d")
        dved1 = sem("dved1")      # L1 out1 stats epilogue done (ACT)
        sq2d = sem("sq2d")        # DVE square for L2 stats
        o2cp = sem("o2cp")        # ACT out2 copies
        stcnt = sem("stcnt")
        st2c = sem("st2c")
        stsr = sem("stsr")
        gs2 = [sem(f"gs2_{i}") for i in range(NBUF)]
        agS, arS, ar2S = sem("agS"), sem("arS"), sem("ar2S")
        cc = sem("cc")
        ar1L, ar2L = sem("ar1L"), sem("ar2L")
        cfa, cfb, cf1 = sem("cfa"), sem("cfb"), sem("cf1")
        cfa2, cfb2, cf2 = sem("cfa2"), sem("cfb2"), sem("cf2")
        cfr = sem("cfr")
        hapA = sem("hapA")        # ACT bn1 apply per window
        hTd = sem("hTd")          # PE transpose done
        hnm = sem("hnm")          # ACT copy to h_nm
        bn2r = sem("bn2r")
        gG = sem("gG")
        plm = sem("plm")
        outc = sem("outc")
        iot = sem("iot")
        bp1, bp2 = sem("bp1"), sem("bp2")
        ioh = sem("ioh")
        cfc = sem("cfc")

        NLOAD = 9
        cfc_n = [0]

        def _chain(v, inst):
            cfc_n[0] += 1
            inst.then_inc(cfc, 1)
            v.wait_ge(cfc, cfc_n[0])

        def _coef_math(v, D, ar_sem, cfa_s, cfb_s, cf_s, bcol, gcol, becol):
            v.wait_ge(ar_sem, 16)
            _chain(v, v.tensor_scalar_mul(tmp_sb[:D, 0:1], stat_sb[:D, 0:1],
                                          1.0 / n_real))
            _chain(v, v.tensor_scalar_mul(tmp_sb[:D, 1:2], stat_sb[:D, 1:2],
                                          1.0 / n_real))
            _chain(v, v.tensor_tensor(out=stat_sb[:D, 0:1],
                                      in0=tmp_sb[:D, 0:1],
                                      in1=tmp_sb[:D, 0:1], op=OP.mult))
            _chain(v, v.tensor_tensor(out=stat_sb[:D, 1:2],
                                      in0=tmp_sb[:D, 1:2],
                                      in1=stat_sb[:D, 0:1], op=OP.subtract))
            v.tensor_scalar_add(stat_sb[:D, 1:2], stat_sb[:D, 1:2],
                                EPS).then_inc(cfa_s, 1)
            v.wait_ge(cfb_s, 1)          # ACT took sqrt in place
            _chain(v, v.reciprocal(out=stat_sb[:D, 1:2],
                                   in_=stat_sb[:D, 1:2]))
            _chain(v, v.tensor_tensor(out=coef_sb[:D, 1:2],
                                      in0=stat_sb[:D, 1:2],
                                      in1=bn_sb[:D, gcol:gcol + 1],
                                      op=OP.mult))   # a
            _chain(v, v.tensor_tensor(out=tmp_sb[:D, 0:1],
                                      in0=tmp_sb[:D, 0:1],
                                      in1=bn_sb[:D, bcol:bcol + 1],
                                      op=OP.add))    # mu
            _chain(v, v.tensor_tensor(out=tmp_sb[:D, 1:2],
                                      in0=tmp_sb[:D, 0:1],
                                      in1=coef_sb[:D, 1:2], op=OP.mult))
            v.tensor_tensor(out=coef_sb[:D, 0:1],
                            in0=bn_sb[:D, becol:becol + 1],
                            in1=tmp_sb[:D, 1:2],
                            op=OP.subtract).then_inc(cf_s, 1)   # bshift

        with nc.Block() as block:

            # ------------------------------------------------ GPSIMD
            @block.gpsimd
            def _(gp: bass.BassGpSimd):
                gp.load_library(mlp)
                gp.iota(iota_sb[:, :], [[1, W]], base=0, channel_multiplier=0,
                        allow_small_or_imprecise_dtypes=True)
                gp.iota(iot4_sb[:, :], [[1, 512]], base=0,
                        channel_multiplier=0,
                        allow_small_or_imprecise_dtypes=True)
                gp.iota(iotg_sb[:, :], [[1, NG]], base=0, channel_multiplier=0,
                        allow_small_or_imprecise_dtypes=True)
                gp.iota(iotp_sb[:, :], [[1, 1]], base=0, channel_multiplier=1,
                        allow_small_or_imprecise_dtypes=True).then_inc(iot, 1)

                # 1/3 of the L1 S builds (gpsimd is idle during L1)
                gp.wait_ge(io, 16 * NLOAD)
                gp.wait_ge(iot, 1)
                for t in range(n_chunks):
                    if t % 3 != 2:
                        continue
                    if t >= 8:
                        gp.wait_ge(pchunk, t - 7)
                    gp.tensor_scalar(
                        out=s1sl[t % 8][:, :], in0=iota_sb[:, :],
                        scalar1=do_sb[:, t:t + 1],
                        scalar2=ew_sb[:, t:t + 1],
                        op0=OP.is_equal, op1=OP.mult).then_inc(sdoneG, 1)

                # BN1 stats AllReduce
                gp.wait_ge(arS, 16)
                gp.collective_compute(
                    "AllReduce", OP.add, replica_groups=[list(range(NC))],
                    ins=[ar1_in[:, :]], outs=[ar1_out[:, :]]).then_inc(cc, 1)
                # h table AllGather
                gp.wait_ge(agS, 16)
                gp.collective_compute(
                    "AllGather", OP.bypass, replica_groups=[list(range(NC))],
                    ins=[ag_in[:, :]], outs=[ag_out[:, :]]).then_inc(cc, 1)

                # L2 gathers (table = AllGather output; wait for completion)
                gp.wait_ge(cc, 2)
                for ci, (cb, nchc) in enumerate(calls):
                    b = ci % NBUF
                    if ci >= NBUF:
                        pcb, pnch = calls[ci - NBUF]
                        gp.wait_ge(pchunk2, pcb + pnch)
                    nidx = nchc * 128
                    gp.dma_gather(
                        mb2[b][:, :nchc, :], ag_out[:, :],
                        idx_sb[:, cb * 8:cb * 8 + nidx // 16],
                        nidx, nidx, 4 * HID,
                    ).then_inc(gs2[b], 16)

                gp.wait_ge(ar2S, 16)
                gp.collective_compute(
                    "AllReduce", OP.add, replica_groups=[list(range(NC))],
                    ins=[ar2_in[:, :]], outs=[ar2_out[:, :]]).then_inc(cc, 1)

            # ------------------------------------------------ VECTOR
            @block.vector
            def _(v):
                v.wait_ge(io, 16 * NLOAD)
                v.wait_ge(iot, 1)
                # identity (fp16) for PE transposes; fp16 msk copy
                _chain(v, v.tensor_scalar(
                    out=id16_sb[:, :], in0=iota_sb[:, :],
                    scalar1=iotp_sb[:, :], scalar2=None, op0=OP.is_equal))
                _chain(v, v.tensor_copy(out=msk_sb[:, :], in_=mskf_sb[:, :]))

                # L1 S builds (2/3 of chunks; t%3==2 built on gpsimd)
                for t in range(n_chunks):
                    if t % 3 == 2:
                        continue
                    if t >= 8:
                        v.wait_ge(pchunk, t - 7)
                    v.tensor_scalar(
                        out=s1sl[t % 8][:, :], in0=iota_sb[:, :],
                        scalar1=do_sb[:, t:t + 1],
                        scalar2=ew_sb[:, t:t + 1],
                        op0=OP.is_equal, op1=OP.mult).then_inc(sdone, 1)

                # BN1 stats reduce + coef math
                v.wait_ge(dved1, NWC)
                _chain(v, v.tensor_reduce(
                    stat_sb[:HID, 0:1], stats1_sb[:, :NWC],
                    axis=mybir.AxisListType.X, op=OP.add))
                v.tensor_reduce(
                    stat_sb[:HID, 1:2], stats1_sb[:, NWC:],
                    axis=mybir.AxisListType.X, op=OP.add).then_inc(stsr, 1)
                _coef_math(v, HID, ar1L, cfa, cfb, cf1, 0, 1, 2)

                # L2 S4 builds + square epilogues
                nxt = 0
                for t in range(n_chunks):
                    if t >= 8:
                        v.wait_ge(pchunk2, t - 7)
                    v.tensor_scalar(
                        out=s4sl[t % 8][:, :], in0=iot4_sb[:, :],
                        scalar1=comb_sb[:, t:t + 1],
                        scalar2=ew_sb[:, t:t + 1],
                        op0=OP.is_equal, op1=OP.mult).then_inc(s4done, 1)
                    while nxt < NWC and nxt + 1 < NWC and t >= wlast[nxt + 1]:
                        w = nxt
                        v.wait_ge(o2cp, w + 1)
                        if w >= 2:
                            v.wait_ge(stcnt, w - 1)   # sq2 ring reuse
                        v.tensor_tensor(
                            out=sq2_sb[w % 2][:, :],
                            in0=out2_sb[:, w * EMB:(w + 1) * EMB],
                            in1=out2_sb[:, w * EMB:(w + 1) * EMB],
                            op=OP.mult).then_inc(sq2d, 1)
                        nxt += 1
                while nxt < NWC:
                    w = nxt
                    v.wait_ge(o2cp, w + 1)
                    if w >= 2:
                        v.wait_ge(stcnt, w - 1)
                    v.tensor_tensor(
                        out=sq2_sb[w % 2][:, :],
                        in0=out2_sb[:, w * EMB:(w + 1) * EMB],
                        in1=out2_sb[:, w * EMB:(w + 1) * EMB],
                        op=OP.mult).then_inc(sq2d, 1)
                    nxt += 1

                _coef_math(v, EMB, ar2L, cfa2, cfb2, cf2, 3, 4, 5)
                v.wait_ge(cf2, 1)
                v.tensor_copy(out=coefh_sb[:EMB, :],
                              in_=coef_sb[:EMB, :]).then_inc(cf2, 1)

                # BN2 apply (node-major, 3 passes) + pool one-hots
                v.wait_ge(cfr, 16 * 2)
                for w in range(NWC):
                    inst = v.tensor_tensor(
                        out=out2_sb[:, w * EMB:(w + 1) * EMB],
                        in0=out2_sb[:, w * EMB:(w + 1) * EMB],
                        in1=coefr_sb[:, EMB:], op=OP.mult)
                inst.then_inc(bp1, 1)
                v.wait_ge(bp1, 1)
                for w in range(NWC):
                    inst = v.tensor_tensor(
                        out=out2_sb[:, w * EMB:(w + 1) * EMB],
                        in0=out2_sb[:, w * EMB:(w + 1) * EMB],
                        in1=coefr_sb[:, :EMB], op=OP.add)
                inst.then_inc(bp2, 1)
                v.wait_ge(bp2, 1)
                for w in range(NWC):
                    v.tensor_scalar_max(
                        out=out2_sb[:, w * EMB:(w + 1) * EMB],
                        in0=out2_sb[:, w * EMB:(w + 1) * EMB],
                        scalar1=0.0).then_inc(bn2r, 1)
                    if w >= 2:
                        v.wait_ge(plm, w - 1)
                    v.tensor_scalar(
                        out=gone_sb[w % 2][:, :], in0=iotg_sb[:, :],
                        scalar1=gid_sb[:, w:w + 1], scalar2=None,
                        op0=OP.is_equal).then_inc(gG, 1)

            # ------------------------------------------------ SCALAR
            @block.scalar
            def _(sc):
                sc.wait_ge(io, 16 * NLOAD)
                # L1 per-window: segx copy, then out1 stats epilogue
                for w in range(NWC):
                    sc.wait_ge(pchunk, wlast[w] + 1)
                    if w >= 2:
                        sc.wait_ge(w1d, w - 1)     # segx_sb ring reuse
                    sc.activation(out=segx_sb[w % 2][:, :],
                                  in_=segq[w % 4][:, :],
                                  func=AF.Copy).then_inc(segcp, 1)
                    sc.wait_ge(w1d, w + 1)
                    sc.activation(out=out1h_sb[:, w * W:(w + 1) * W],
                                  in_=out1_ps[w % 2][:, :], func=AF.Copy,
                                  accum_out=stats1_sb[:, w:w + 1])
                    sc.activation(out=sq_sb[w % 2][:HID, :W],
                                  in_=out1_ps[w % 2][:, :], func=AF.Square,
                                  accum_out=stats1_sb[:, NWC + w:NWC + w + 1]
                                  ).then_inc(dved1, 1)
                # sqrt for BN1
                sc.wait_ge(cfa, 1)
                sc.activation(out=stat_sb[:HID, 1:2], in_=stat_sb[:HID, 1:2],
                              func=AF.Sqrt).then_inc(cfb, 1)
                # BN1 apply per window (fused relu(a*x+b)) -> happ
                sc.wait_ge(cf1, 1)
                for w in range(NWC):
                    if w >= 2:
                        sc.wait_ge(hTd, w - 1)     # happ ring reuse
                    sc.activation(out=happ_sb[w % 2][:, :],
                                  in_=out1h_sb[:, w * W:(w + 1) * W],
                                  func=AF.Relu, bias=coef_sb[:HID, 0:1],
                                  scale=coef_sb[:HID, 1:2]).then_inc(hapA, 1)
                    # copy transpose result of previous window
                    sc.wait_ge(hTd, w + 1)
                    sc.activation(out=h_nm[:, w, :],
                                  in_=hT_ps[w % 2][:, :],
                                  func=AF.Copy).then_inc(hnm, 1)
                # L2 per-window: seg2f copy, out2 copy
                for w in range(NWC):
                    sc.wait_ge(pchunk2, wlast[w] + 1)
                    if w >= 2:
                        sc.wait_ge(w2d, w - 1)
                    sc.activation(out=s2f_sb[w % 2][:, :],
                                  in_=segq[w % 4][:HID, :],
                                  func=AF.Copy).then_inc(seg2cp, 1)
                    sc.wait_ge(w2d, w + 1)
                    sc.activation(out=out2_sb[:, w * EMB:(w + 1) * EMB],
                                  in_=out2_ps[w % 2][:, :],
                                  func=AF.Copy).then_inc(o2cp, 1)
                # L2 stats to sbuf, sqrt
                sc.wait_ge(stcnt, NWC)
                sc.activation(out=stat_sb[:EMB, 0:1], in_=st_ps[0][:EMB, :],
                              func=AF.Copy)
                sc.activation(out=stat_sb[:EMB, 1:2], in_=st_ps[1][:EMB, :],
                              func=AF.Copy).then_inc(st2c, 1)
                sc.wait_ge(cfa2, 1)
                sc.activation(out=stat_sb[:EMB, 1:2], in_=stat_sb[:EMB, 1:2],
                              func=AF.Sqrt).then_inc(cfb2, 1)
                # final pool copies
                sc.wait_ge(plm, NWC)
                for gh in range(GHALF):
                    a = sc.activation(out=pout_sb[:, gh * EMB:(gh + 1) * EMB],
                                      in_=pool_ps[gh][:, :], func=AF.Copy)
                    if gh == GHALF - 1:
                        a.then_inc(outc, 1)

            # ------------------------------------------------ TENSOR
            @block.tensor
            def _(pe):
                pe.wait_ge(io, 16 * NLOAD)
                # L1 chunk matmuls + per-window W1
                done_w1 = 0

                def drain_w1(upto):
                    nonlocal done_w1
                    while done_w1 < upto:
                        w = done_w1
                        pe.wait_ge(segcp, w + 1)
                        if w >= 1:
                            pe.wait_ge(dved1, w)   # out1_ps ring hazard
                        pe.matmul(out1_ps[w % 2][:, :], lhsT=w1_sb[:, :],
                                  rhs=segx_sb[w % 2][:, :], start=True,
                                  stop=True).then_inc(w1d, 1)
                        done_w1 += 1

                for t in range(n_chunks):
                    w = chunk_window[t]
                    sl = t // SLAB
                    pe.wait_ge(xls, 16 * (sl + 1))
                    if t % 3 == 2:
                        pe.wait_ge(sdoneG, (t + 1) // 3)
                    else:
                        pe.wait_ge(sdone, t + 1 - (t + 1) // 3)
                    first, lastc = t == wfirst[w], t == wlast[w]
                    if first and w >= 4:
                        pe.wait_ge(segcp, w - 3)   # psum bank ring
                    pe.matmul(segq[w % 4][:, :],
                              lhsT=xs[sl % 2][:, t - sl * SLAB, :],
                              rhs=s1sl[t % 8][:, :],
                              start=first, stop=lastc).then_inc(pchunk, 1)
                    if lastc:
                        drain_w1(w)    # drain previous windows
                drain_w1(NWC)

                # BN1 transposes
                for w in range(NWC):
                    pe.wait_ge(hapA, w + 1)
                    if w >= 2:
                        pe.wait_ge(hnm, w - 1)
                    pe.transpose(out=hT_ps[w % 2][:, :],
                                 in_=happ_sb[w % 2][:, :],
                                 identity=id16_sb[:HID, :HID]).then_inc(
                                     hTd, 1)

                # L2 chunk matmuls + per-window W2 + stats
                done_w2 = 0
                done_st = 0

                def drain_l2(upto_w2, upto_st):
                    nonlocal done_w2, done_st
                    while done_w2 < upto_w2:
                        w = done_w2
                        pe.wait_ge(seg2cp, w + 1)
                        if w >= 1:
                            pe.wait_ge(o2cp, w)    # out2_ps ring hazard
                        pe.matmul(out2_ps[w % 2][:, :],
                                  lhsT=s2f_sb[w % 2][:, :],
                                  rhs=w2_sb[:, :], start=True,
                                  stop=True).then_inc(w2d, 1)
                        done_w2 += 1
                    while done_st < upto_st:
                        w = done_st
                        pe.wait_ge(sq2d, w + 1)
                        pe.matmul(st_ps[0][:EMB, :],
                                  lhsT=out2_sb[:, w * EMB:(w + 1) * EMB],
                                  rhs=msk_sb[:, w:w + 1],
                                  start=(w == 0), stop=False)
                        pe.matmul(st_ps[1][:EMB, :],
                                  lhsT=sq2_sb[w % 2][:, :],
                                  rhs=msk_sb[:, w:w + 1],
                                  start=False,
                                  stop=(w == NWC - 1)).then_inc(stcnt, 1)
                        done_st += 1

                uses = [0] * NBUF
                for ci, (cb, nchc) in enumerate(calls):
                    b = ci % NBUF
                    uses[b] += 1
                    pe.wait_ge(gs2[b], 16 * uses[b])
                    for k in range(nchc):
                        t = cb + k
                        w = chunk_window[t]
                        pe.wait_ge(s4done, t + 1)
                        first, lastc = t == wfirst[w], t == wlast[w]
                        if t == wfirst[w] and w >= 4:
                            pe.wait_ge(seg2cp, w - 3)
                        for q in range(4):
                            mm = pe.matmul(
                                segq[w % 4][:HID, :],
                                lhsT=mb2[b][:, k, q * HID:(q + 1) * HID],
                                rhs=s4sl[t % 8][:, q * 128:(q + 1) * 128],
                                start=(first and q == 0),
                                stop=(lastc and q == 3))
                            if q == 3:
                                mm.then_inc(pchunk2, 1)
                        if lastc:
                            drain_l2(w, max(0, w - 1))
                drain_l2(NWC, NWC)

                # pool matmuls
                for w in range(NWC):
                    pe.wait_ge(bn2r, w + 1)
                    pe.wait_ge(gG, w + 1)
                    for gh in range(GHALF):
                        mm = pe.matmul(
                            pool_ps[gh][:, :],
                            lhsT=gone_sb[w % 2][:, gh * 128:(gh + 1) * 128],
                            rhs=out2_sb[:, w * EMB:(w + 1) * EMB],
                            start=(w == 0 and gh == 0),
                            stop=(w == NWC - 1 and gh == GHALF - 1))
                        if gh == GHALF - 1:
                            mm.then_inc(plm, 1)

            # ------------------------------------------------ SYNC
            @block.sync
            def _(sy):
                for dst_ap, src_ap in (
                    (idx_sb[:, :], idx_d[:, :]),
                    (comb_sb[:, :], comb_d[:, :]),
                    (do_sb[:, :], do_d[:, :]),
                    (ew_sb[:, :], ew_d[:, :]),
                    (gid_sb[:, :], gid_d[:, :]),
                    (mskf_sb[:, :], msk_d[:, :]),
                    (w1_sb[:, :], w1_d[:, :]),
                    (w2_sb[:, :], w2_d[:, :]),
                    (bn_sb[:, :], bn_d[:, :]),
                ):
                    sy.dma_start(dst_ap, src_ap).then_inc(io, 16)
                # xg slab loads
                for sl in range(n_slabs):
                    if sl >= 2:
                        sy.wait_ge(pchunk, (sl - 1) * SLAB)
                    t0 = sl * SLAB
                    t1 = min(n_chunks, t0 + SLAB)
                    sy.dma_start(xs[sl % 2][:, :t1 - t0, :],
                                 xg_d[:, t0:t1, :]).then_inc(xls, 16)
                # BN1 stats -> AR1
                sy.wait_ge(stsr, 1)
                sy.dma_start(ar1_in[:, :], stat_sb[:HID, 0:2]).then_inc(
                    arS, 16)
                sy.wait_ge(cc, 1)
                sy.dma_start(stat_sb[:HID, 0:2], ar1_out[:, :]).then_inc(
                    ar1L, 16)
                # h table -> AG
                sy.wait_ge(hnm, NWC)
                sy.dma_start(
                    ag_in[:, :].rearrange("(t g) (m d) -> (g m) t d",
                                          g=32, m=4),
                    h_nm[:, :, :]).then_inc(agS, 16)
                # AR2
                sy.wait_ge(st2c, 1)
                sy.dma_start(ar2_in[:, :], stat_sb[:EMB, 0:2]).then_inc(
                    ar2S, 16)
                sy.wait_ge(cc, 3)
                sy.dma_start(stat_sb[:EMB, 0:2], ar2_out[:, :]).then_inc(
                    ar2L, 16)
                # bn2 coef rows -> replicated
                sy.wait_ge(cf2, 2)
                with nc.allow_non_contiguous_dma(reason="tiny 256-elem coef"):
                    sy.dma_start(bnrow[:, :].rearrange("c p -> p c"),
                                 coefh_sb[:EMB, 0:2]).then_inc(cfr, 16)
                sy.wait_ge(cfr, 16)
                rep = bass.AP(bnrow, 0, [[0, 128], [1, 2 * EMB]])
                sy.dma_start(coefr_sb[:, :], rep).then_inc(cfr, 16)
                # final output
                sy.wait_ge(outc, 1)
                sy.dma_start(
                    out_d[:, :, :].rearrange("g p d -> p g d"),
                    pout_sb[:, :].rearrange("p (g d) -> p g d", d=EMB),
                ).then_inc(ioh, 16)
                sy.wait_ge(ioh, 16)

    nc.compile()
    return nc


# ==================================================================== entry
def _make_in_maps(inputs, cfg, percore):
    HID, EMB = cfg["hid_dim"], cfg["emb_dim"]
    bnp = np.zeros((128, 6), np.float32)
    bnp[:HID, 0] = np.asarray(inputs["b1"], np.float32)
    bnp[:HID, 1] = np.asarray(inputs["g1"], np.float32)
    bnp[:HID, 2] = np.asarray(inputs["be1"], np.float32)
    bnp[:EMB, 3] = np.asarray(inputs["b2"], np.float32)
    bnp[:EMB, 4] = np.asarray(inputs["g2"], np.float32)
    bnp[:EMB, 5] = np.asarray(inputs["be2"], np.float32)
    w1 = np.asarray(inputs["W1"], np.float32).astype(np.float16)
    w2 = np.asarray(inputs["W2"], np.float32).astype(np.float16)
    return [dict(
        xg=percore["xg"][c], idx=percore["idx"][c], cmb=percore["comb"][c],
        dof=percore["dstoff"][c], ewt=percore["ew"][c],
        gid=percore["gid"][c], msk=percore["msk"][c], w1=w1, w2=w2, bnp=bnp,
    ) for c in range(cfg["n_cores"])]


def _run(inputs, cfg):
    x = np.asarray(inputs["x"], np.float32)
    layout, percore, slot = _host_prep(
        x, inputs["edge_index"], inputs["edge_weight"], inputs["batch_vec"],
        cfg)
    nc = _build(cfg, layout)

    NC = cfg["n_cores"]
    in_maps = _make_in_maps(inputs, cfg, percore)
    res = run_bass_kernel_spmd(nc, in_maps, list(range(NC)), trace=TRACE)

    NG, EMB = cfg["n_graphs"], cfg["emb_dim"]
    pool = np.zeros((NG, EMB), np.float64)
    for c in range(NC):
        p = res.results[c]["pool"].astype(np.float64)
        pool += p.reshape(NG, EMB)
    counts = np.bincount(np.asarray(inputs["batch_vec"], np.int64),
                         minlength=NG).astype(np.float64)
    pool /= np.maximum(counts, 1.0)[:, None]
    return pool.astype(np.float32), res


def kernel(**inputs):
    out, _ = _run(inputs, CFG_FULL)
    return out


# revision 7
# speedup vs baseline: 1.3630x; 1.3630x over previous
"""GCN encoder on 8 trn2 cores, v2.

Key changes vs v1:
  - L1 messages are host-expanded (xg = x[src] fp16, per-edge, chunk layout)
    and STREAMED via affine HWDGE DMA - no per-edge descriptor generation.
  - L2 gathers h rows via dma_gather of 4-node groups (512B descs, int16
    idx = slot>>2, no range buckets); group-slot selection is fused into a
    512-wide one-hot S4 so each 128-edge chunk costs 1 DVE build + 4 PE
    matmuls.
  - Windows are chunk-contiguous (no WG interleave), caps degree-balanced.
"""
import numpy as np
from contextlib import ExitStack

import concourse.bacc as bacc
import concourse.bass as bass
import concourse.mybir as mybir
from concourse.bass_utils import run_bass_kernel_spmd
from concourse.library_config import mlp

F32 = mybir.dt.float32
F16 = mybir.dt.float16
I16 = mybir.dt.int16
AF = mybir.ActivationFunctionType
OP = mybir.AluOpType

EPS = 1e-5
TRACE = False

CFG_FULL = dict(n_nodes=100000, n_edges=1600000, n_cores=8,
                slots_per_core=12544, in_dim=128, hid_dim=64, emb_dim=128,
                n_graphs=256)


# ================================================================ host prep
def _degree_balanced_perm(dst, n_nodes, n_windows, wsize):
    import heapq
    deg = np.bincount(dst, minlength=n_nodes)
    order = np.argsort(-deg, kind="stable")
    heap = [(0, w) for w in range(n_windows)]
    heapq.heapify(heap)
    counts = np.zeros(n_windows, np.int64)
    slot = np.empty(n_nodes, np.int64)
    degs = deg[order]
    for i in range(n_nodes):
        load, w = heapq.heappop(heap)
        slot[order[i]] = w * wsize + counts[w]
        counts[w] += 1
        if counts[w] < wsize:
            heapq.heappush(heap, (load + int(degs[i]), w))
    return slot


def _wrap16(flat):
    n = flat.size
    w = flat.reshape(n // 16, 16).T.astype(np.int16)
    return np.tile(w, (8, 1))


def _host_prep(x, edge_index, edge_weight, batch_vec, cfg):
    NC, SPC = cfg["n_cores"], cfg["slots_per_core"]
    W = 128
    NWC = SPC // W
    IN = cfg["in_dim"]
    n_nodes = cfg["n_nodes"]

    src = np.asarray(edge_index[0], np.int64)
    dst = np.asarray(edge_index[1], np.int64)
    ew = np.asarray(edge_weight, np.float32)

    slot = _degree_balanced_perm(dst, n_nodes, NC * NWC, W)

    sslot, dslot = slot[src], slot[dst]
    core = dslot // SPC
    wloc = (dslot % SPC) // W
    dstoff = (dslot % W).astype(np.float32)
    grp = (sslot >> 2).astype(np.int64)
    comb = (dstoff + 128.0 * (sslot & 3)).astype(np.float32)

    # caps per window = max over cores, rounded to 128
    key = core * NWC + wloc
    cnt = np.bincount(key, minlength=NC * NWC).reshape(NC, NWC)
    caps = np.maximum(128, ((cnt.max(axis=0) + 127) // 128) * 128)  # [NWC]

    nch_w = caps // 128
    wchunk0 = np.concatenate([[0], np.cumsum(nch_w)])
    n_chunks = int(wchunk0[-1])
    chunk_window = np.repeat(np.arange(NWC), nch_w)
    wfirst = wchunk0[:-1]
    wlast = wchunk0[1:] - 1

    calls = []
    k = 0
    while k < n_chunks:
        n = min(8, n_chunks - k)
        calls.append((k, n))
        k += n

    x16 = np.asarray(x, np.float16)
    idx_cores, comb_cores, ew_cores, do_cores, xg_cores = [], [], [], [], []
    for c in range(NC):
        m = core == c
        sg, cb, dv, wv, wgt, sr = (grp[m], comb[m], dstoff[m], wloc[m],
                                   ew[m], src[m])
        e_idx = np.zeros(n_chunks * 128, np.int64)
        e_cb = np.zeros(n_chunks * 128, np.float32)
        e_do = np.zeros(n_chunks * 128, np.float32)
        e_ew = np.zeros(n_chunks * 128, np.float32)
        e_src = np.zeros(n_chunks * 128, np.int64)
        for w in range(NWC):
            sel = wv == w
            n = int(sel.sum())
            s = int(wfirst[w]) * 128
            e_idx[s:s + n] = sg[sel]
            e_cb[s:s + n] = cb[sel]
            e_do[s:s + n] = dv[sel]
            e_ew[s:s + n] = wgt[sel]
            e_src[s:s + n] = sr[sel]
        idx_cores.append(_wrap16(e_idx))
        comb_cores.append(np.ascontiguousarray(
            e_cb.reshape(n_chunks, 128).T))
        do_cores.append(np.ascontiguousarray(
            e_do.reshape(n_chunks, 128).T))
        ew_cores.append(np.ascontiguousarray(
            e_ew.reshape(n_chunks, 128).T))
        # xg[p, t, :] = x[src of edge at chunk t partition p]  (ew=0 rows pad)
        xg = x16[e_src].reshape(n_chunks, 128, IN).transpose(1, 0, 2)
        xg_cores.append(np.ascontiguousarray(xg))

    gid = np.full(NC * SPC, -1.0, np.float32)
    gid[slot] = np.asarray(batch_vec, np.float32)
    msk = np.zeros(NC * SPC, np.float32)
    msk[slot] = 1.0
    gid_cores = [np.ascontiguousarray(
        gid[c * SPC:(c + 1) * SPC].reshape(NWC, W).T) for c in range(NC)]
    msk_cores = [np.ascontiguousarray(
        msk[c * SPC:(c + 1) * SPC].reshape(NWC, W).T) for c in range(NC)]

    layout = dict(n_chunks=n_chunks, calls=calls, NWC=NWC,
                  chunk_window=chunk_window.tolist(),
                  wfirst=wfirst.tolist(), wlast=wlast.tolist())
    percore = dict(idx=idx_cores, comb=comb_cores, dstoff=do_cores,
                   ew=ew_cores, xg=xg_cores, gid=gid_cores, msk=msk_cores)
    return layout, percore, slot


# ============================================================= bass program
def _build(cfg, layout):
    NC, SPC = cfg["n_cores"], cfg["slots_per_core"]
    IN, HID, EMB = cfg["in_dim"], cfg["hid_dim"], cfg["emb_dim"]
    NG = cfg["n_graphs"]
    NSLOT = NC * SPC
    NWC = layout["NWC"]
    W = 128
    n_chunks = layout["n_chunks"]
    calls = layout["calls"]
    chunk_window = layout["chunk_window"]
    wfirst, wlast = layout["wfirst"], layout["wlast"]
    n_real = cfg["n_nodes"]
    GHALF = NG // 128
    NBUF = 3
    ncalls = len(calls)
    SLAB = 64
    n_slabs = (n_chunks + SLAB - 1) // SLAB
    NGRP = NSLOT // 4          # 4-node groups in gather table

    nc = bacc.Bacc("TRN2")

    xg_d = nc.dram_tensor("xg", [128, n_chunks, IN], F16, kind="ExternalInput")
    idx_d = nc.dram_tensor("idx", [128, n_chunks * 8], I16,
                           kind="ExternalInput")
    comb_d = nc.dram_tensor("cmb", [128, n_chunks], F32, kind="ExternalInput")
    do_d = nc.dram_tensor("dof", [128, n_chunks], F32, kind="ExternalInput")
    ew_d = nc.dram_tensor("ewt", [128, n_chunks], F32, kind="ExternalInput")
    gid_d = nc.dram_tensor("gid", [128, NWC], F32, kind="ExternalInput")
    msk_d = nc.dram_tensor("msk", [128, NWC], F32, kind="ExternalInput")
    w1_d = nc.dram_tensor("w1", [IN, HID], F16, kind="ExternalInput")
    w2_d = nc.dram_tensor("w2", [HID, EMB], F16, kind="ExternalInput")
    bn_d = nc.dram_tensor("bnp", [128, 6], F32, kind="ExternalInput")
    out_d = nc.dram_tensor("pool", [GHALF, 128, EMB], F32,
                           kind="ExternalOutput")

    ag_in = nc.dram_tensor("ag_in", [SPC // 4, 4 * HID], F16)
    ag_out = nc.dram_tensor("ag_out", [NGRP, 4 * HID], F16,
                            addr_space="Shared")
    ar1_in = nc.dram_tensor("ar1_in", [HID, 2], F32)
    ar1_out = nc.dram_tensor("ar1_out", [HID, 2], F32, addr_space="Shared")
    ar2_in = nc.dram_tensor("ar2_in", [EMB, 2], F32)
    ar2_out = nc.dram_tensor("ar2_out", [EMB, 2], F32, addr_space="Shared")
    bnrow = nc.dram_tensor("bnrow", [2, EMB], F16)

    with ExitStack() as ctx:
        sb = lambda n, s, d: ctx.enter_context(nc.sbuf_tensor(n, s, d))
        sem = lambda n: ctx.enter_context(nc.semaphore(n))

        idx_sb = sb("idx_sb", [128, n_chunks * 8], I16)
        comb_sb = sb("comb_sb", [128, n_chunks], F32)
        do_sb = sb("do_sb", [128, n_chunks], F32)
        ew_sb = sb("ew_sb", [128, n_chunks], F32)
        gid_sb = sb("gid_sb", [128, NWC], F32)
        msk_sb = sb("msk_sb", [128, NWC], F16)
        mskf_sb = sb("mskf_sb", [128, NWC], F32)
        w1_sb = sb("w1_sb", [IN, HID], F16)
        w2_sb = sb("w2_sb", [HID, EMB], F16)
        bn_sb = sb("bn_sb", [128, 6], F32)
        iota_sb = sb("iota_sb", [128, W], F16)
        iot4_sb = sb("iot4_sb", [128, 512], F16)
        iotg_sb = sb("iotg_sb", [128, NG], F16)
        iotp_sb = sb("iotp_sb", [128, 1], F32)
        id16_sb = sb("id16_sb", [128, 128], F16)

        xs = [sb(f"xs_{i}", [128, SLAB, IN], F16) for i in range(2)]
        s1sl = [sb(f"s1_{i}", [128, W], F16) for i in range(8)]
        s4sl = [sb(f"s4_{i}", [128, 512], F16) for i in range(8)]
        mb2 = [sb(f"mb2_{i}", [128, 8, 4 * HID], F16) for i in range(NBUF)]
        segx_sb = [sb(f"sgx_{i}", [128, W], F16) for i in range(2)]
        s2f_sb = [sb(f"s2f_{i}", [HID, W], F16) for i in range(2)]
        happ_sb = [sb(f"hap_{i}", [HID, W], F16) for i in range(2)]
        sq_sb = [sb(f"sq_{i}", [128, W], F32) for i in range(2)]
        sq2_sb = [sb(f"sq2_{i}", [128, W], F16) for i in range(2)]
        out1h_sb = sb("out1h_sb", [HID, NWC * W], F16)
        stats1_sb = sb("stats1_sb", [HID, 2 * NWC], F32)
        h_nm = sb("h_nm", [128, NWC, HID], F16)
        out2_sb = sb("out2_sb", [128, NWC * EMB], F16)
        stat_sb = sb("stat_sb", [128, 2], F32)
        tmp_sb = sb("tmp_sb", [128, 2], F32)
        coef_sb = sb("coef_sb", [128, 2], F32)
        coefh_sb = sb("coefh_sb", [128, 2], F16)
        coefr_sb = sb("coefr_sb", [128, 2 * EMB], F16)
        gone_sb = [sb(f"gone_{i}", [128, NG], F16) for i in range(2)]
        pout_sb = sb("pout_sb", [128, GHALF * EMB], F32)

        # psum: one full bank per in-flight seg window (ring of 4); long
        # accumulation groups (stats, pool) share b4, read only at the end.
        sgt = [ctx.enter_context(nc.psum_tensor(f"sg{i}", [128, 512], F32))
               for i in range(4)]
        segq = [t[:, 0:128] for t in sgt]
        b2 = ctx.enter_context(nc.psum_tensor("b2", [128, 512], F32))
        out1_ps = [b2[:HID, 0:W], b2[:HID, W:2 * W]]
        b5 = ctx.enter_context(nc.psum_tensor("b5", [128, 1024], F16))
        hT_ps = [b5[:, 0:HID], b5[:, HID:2 * HID]]
        b3 = ctx.enter_context(nc.psum_tensor("b3", [128, 512], F32))
        out2_ps = [b3[:, 0:EMB], b3[:, EMB:2 * EMB]]
        b4 = ctx.enter_context(nc.psum_tensor("b4", [128, 512], F32))
        pool_ps = [b4[:, i * EMB:(i + 1) * EMB] for i in range(GHALF)]
        st_ps = [b4[:, 2 * EMB:2 * EMB + 1], b4[:, 2 * EMB + 1:2 * EMB + 2]]

        io = sem("io")
        xls = sem("xls")
        sdone = sem("sdone")      # S1 builds
        s4done = sem("s4done")    # S4 builds
        pchunk = sem("pchunk")    # L1 chunk matmuls
        pchunk2 = sem("pchunk2")  # L2 chunk matmuls (1 per chunk)
        segcp = sem("segcp")      # ACT segx copies (L1)
        seg2cp = sem("seg2cp")    # ACT seg2f copies (L2)
        w1d = sem("w1d")
        w2d = sem("w2d")
        dved1 = sem("dved1")      # L1 out1 stats epilogue done (ACT)
        sq2d = sem("sq2d")        # DVE square for L2 stats
        o2cp = sem("o2cp")        # ACT out2 copies
        stcnt = sem("stcnt")
        st2c = sem("st2c")
        stsr = sem("stsr")
        gs2 = [sem(f"gs2_{i}") for i in range(NBUF)]
        agS, arS, ar2S = sem("agS"), sem("arS"), sem("ar2S")
        cc = sem("cc")
        ar1L, ar2L = sem("ar1L"), sem("ar2L")
        cfa, cfb, cf1 = sem("cfa"), sem("cfb"), sem("cf1")
        cfa2, cfb2, cf2 = sem("cfa2"), sem("cfb2"), sem("cf2")
        cfr = sem("cfr")
        hapA = sem("hapA")        # ACT bn1 apply per window
        hTd = sem("hTd")          # PE transpose done
        hnm = sem("hnm")          # ACT copy to h_nm
        bn2r = sem("bn2r")
        gG = sem("gG")
        plm = sem("plm")
        outc = sem("outc")
        iot = sem("iot")
        bp1, bp2 = sem("bp1"), sem("bp2")
        ioh = sem("ioh")
        cfc = sem("cfc")

        NLOAD = 9
        cfc_n = [0]

        def _chain(v, inst):
            cfc_n[0] += 1
            inst.then_inc(cfc, 1)
            v.wait_ge(cfc, cfc_n[0])

        def _coef_math(v, D, ar_sem, cfa_s, cfb_s, cf_s, bcol, gcol, becol):
            v.wait_ge(ar_sem, 16)
            _chain(v, v.tensor_scalar_mul(tmp_sb[:D, 0:1], stat_sb[:D, 0:1],
                                          1.0 / n_real))
            _chain(v, v.tensor_scalar_mul(tmp_sb[:D, 1:2], stat_sb[:D, 1:2],
                                          1.0 / n_real))
            _chain(v, v.tensor_tensor(out=stat_sb[:D, 0:1],
                                      in0=tmp_sb[:D, 0:1],
                                      in1=tmp_sb[:D, 0:1], op=OP.mult))
            _chain(v, v.tensor_tensor(out=stat_sb[:D, 1:2],
                                      in0=tmp_sb[:D, 1:2],
                                      in1=stat_sb[:D, 0:1], op=OP.subtract))
            v.tensor_scalar_add(stat_sb[:D, 1:2], stat_sb[:D, 1:2],
                                EPS).then_inc(cfa_s, 1)
            v.wait_ge(cfb_s, 1)          # ACT took sqrt in place
            _chain(v, v.reciprocal(out=stat_sb[:D, 1:2],
                                   in_=stat_sb[:D, 1:2]))
            _chain(v, v.tensor_tensor(out=coef_sb[:D, 1:2],
                                      in0=stat_sb[:D, 1:2],
                                      in1=bn_sb[:D, gcol:gcol + 1],
                                      op=OP.mult))   # a
            _chain(v, v.tensor_tensor(out=tmp_sb[:D, 0:1],
                                      in0=tmp_sb[:D, 0:1],
                                      in1=bn_sb[:D, bcol:bcol + 1],
                                      op=OP.add))    # mu
            _chain(v, v.tensor_tensor(out=tmp_sb[:D, 1:2],
                                      in0=tmp_sb[:D, 0:1],
                                      in1=coef_sb[:D, 1:2], op=OP.mult))
            v.tensor_tensor(out=coef_sb[:D, 0:1],
                            in0=bn_sb[:D, becol:becol + 1],
                            in1=tmp_sb[:D, 1:2],
                            op=OP.subtract).then_inc(cf_s, 1)   # bshift

        with nc.Block() as block:

            # ------------------------------------------------ GPSIMD
            @block.gpsimd
            def _(gp: bass.BassGpSimd):
                gp.load_library(mlp)
                gp.iota(iota_sb[:, :], [[1, W]], base=0, channel_multiplier=0,
                        allow_small_or_imprecise_dtypes=True)
                gp.iota(iot4_sb[:, :], [[1, 512]], base=0,
                        channel_multiplier=0,
                        allow_small_or_imprecise_dtypes=True)
                gp.iota(iotg_sb[:, :], [[1, NG]], base=0, channel_multiplier=0,
                        allow_small_or_imprecise_dtypes=True)
                gp.iota(iotp_sb[:, :], [[1, 1]], base=0, channel_multiplier=1,
                        allow_small_or_imprecise_dtypes=True).then_inc(iot, 1)

                # BN1 stats AllReduce
                gp.wait_ge(arS, 16)
                gp.collective_compute(
                    "AllReduce", OP.add, replica_groups=[list(range(NC))],
                    ins=[ar1_in[:, :]], outs=[ar1_out[:, :]]).then_inc(cc, 1)
                # h table AllGather
                gp.wait_ge(agS, 16)
                gp.collective_compute(
                    "AllGather", OP.bypass, replica_groups=[list(range(NC))],
                    ins=[ag_in[:, :]], outs=[ag_out[:, :]]).then_inc(cc, 1)

                # L2 gathers (table = AllGather output; wait for completion)
                gp.wait_ge(cc, 2)
                for ci, (cb, nchc) in enumerate(calls):
                    b = ci % NBUF
                    if ci >= NBUF:
                        pcb, pnch = calls[ci - NBUF]
                        gp.wait_ge(pchunk2, pcb + pnch)
                    nidx = nchc * 128
                    gp.dma_gather(
                        mb2[b][:, :nchc, :], ag_out[:, :],
                        idx_sb[:, cb * 8:cb * 8 + nidx // 16],
                        nidx, nidx, 4 * HID,
                    ).then_inc(gs2[b], 16)

                gp.wait_ge(ar2S, 16)
                gp.collective_compute(
                    "AllReduce", OP.add, replica_groups=[list(range(NC))],
                    ins=[ar2_in[:, :]], outs=[ar2_out[:, :]]).then_inc(cc, 1)

            # ------------------------------------------------ VECTOR
            @block.vector
            def _(v):
                v.wait_ge(io, 16 * NLOAD)
                v.wait_ge(iot, 1)
                # identity (fp16) for PE transposes; fp16 msk copy
                _chain(v, v.tensor_scalar(
                    out=id16_sb[:, :], in0=iota_sb[:, :],
                    scalar1=iotp_sb[:, :], scalar2=None, op0=OP.is_equal))
                _chain(v, v.tensor_copy(out=msk_sb[:, :], in_=mskf_sb[:, :]))

                # L1 S builds
                for t in range(n_chunks):
                    if t >= 8:
                        v.wait_ge(pchunk, t - 7)
                    v.tensor_scalar(
                        out=s1sl[t % 8][:, :], in0=iota_sb[:, :],
                        scalar1=do_sb[:, t:t + 1],
                        scalar2=ew_sb[:, t:t + 1],
                        op0=OP.is_equal, op1=OP.mult).then_inc(sdone, 1)

                # BN1 stats reduce + coef math
                v.wait_ge(dved1, NWC)
                _chain(v, v.tensor_reduce(
                    stat_sb[:HID, 0:1], stats1_sb[:, :NWC],
                    axis=mybir.AxisListType.X, op=OP.add))
                v.tensor_reduce(
                    stat_sb[:HID, 1:2], stats1_sb[:, NWC:],
                    axis=mybir.AxisListType.X, op=OP.add).then_inc(stsr, 1)
                _coef_math(v, HID, ar1L, cfa, cfb, cf1, 0, 1, 2)

                # L2 S4 builds + square epilogues
                nxt = 0
                for t in range(n_chunks):
                    if t >= 8:
                        v.wait_ge(pchunk2, t - 7)
                    v.tensor_scalar(
                        out=s4sl[t % 8][:, :], in0=iot4_sb[:, :],
                        scalar1=comb_sb[:, t:t + 1],
                        scalar2=ew_sb[:, t:t + 1],
                        op0=OP.is_equal, op1=OP.mult).then_inc(s4done, 1)
                    while nxt < NWC and nxt + 1 < NWC and t >= wlast[nxt + 1]:
                        w = nxt
                        v.wait_ge(o2cp, w + 1)
                        if w >= 2:
                            v.wait_ge(stcnt, w - 1)   # sq2 ring reuse
                        v.tensor_tensor(
                            out=sq2_sb[w % 2][:, :],
                            in0=out2_sb[:, w * EMB:(w + 1) * EMB],
                            in1=out2_sb[:, w * EMB:(w + 1) * EMB],
                            op=OP.mult).then_inc(sq2d, 1)
                        nxt += 1
                while nxt < NWC:
                    w = nxt
                    v.wait_ge(o2cp, w + 1)
                    if w >= 2:
                        v.wait_ge(stcnt, w - 1)
                    v.tensor_tensor(
                        out=sq2_sb[w % 2][:, :],
                        in0=out2_sb[:, w * EMB:(w + 1) * EMB],
                        in1=out2_sb[:, w * EMB:(w + 1) * EMB],
                        op=OP.mult).then_inc(sq2d, 1)
                    nxt += 1

                _coef_math(v, EMB, ar2L, cfa2, cfb2, cf2, 3, 4, 5)
                v.wait_ge(cf2, 1)
                v.tensor_copy(out=coefh_sb[:EMB, :],
                              in_=coef_sb[:EMB, :]).then_inc(cf2, 1)

                # BN2 apply (node-major, 3 passes) + pool one-hots
                v.wait_ge(cfr, 16 * 2)
                for w in range(NWC):
                    inst = v.tensor_tensor(
                        out=out2_sb[:, w * EMB:(w + 1) * EMB],
                        in0=out2_sb[:, w * EMB:(w + 1) * EMB],
                        in1=coefr_sb[:, EMB:], op=OP.mult)
                inst.then_inc(bp1, 1)
                v.wait_ge(bp1, 1)
                for w in range(NWC):
                    inst = v.tensor_tensor(
                        out=out2_sb[:, w * EMB:(w + 1) * EMB],
                        in0=out2_sb[:, w * EMB:(w + 1) * EMB],
                        in1=coefr_sb[:, :EMB], op=OP.add)
                inst.then_inc(bp2, 1)
                v.wait_ge(bp2, 1)
                for w in range(NWC):
                    v.tensor_scalar_max(
                        out=out2_sb[:, w * EMB:(w + 1) * EMB],
                        in0=out2_sb[:, w * EMB:(w + 1) * EMB],
                        scalar1=0.0).then_inc(bn2r, 1)
                    if w >= 2:
                        v.wait_ge(plm, w - 1)
                    v.tensor_scalar(
                        out=gone_sb[w % 2][:, :], in0=iotg_sb[:, :],
                        scalar1=gid_sb[:, w:w + 1], scalar2=None,
                        op0=OP.is_equal).then_inc(gG, 1)

            # ------------------------------------------------ SCALAR
            @block.scalar
            def _(sc):
                sc.wait_ge(io, 16 * NLOAD)
                # L1 per-window: segx copy, then out1 stats epilogue
                for w in range(NWC):
                    sc.wait_ge(pchunk, wlast[w] + 1)
                    if w >= 2:
                        sc.wait_ge(w1d, w - 1)     # segx_sb ring reuse
                    sc.activation(out=segx_sb[w % 2][:, :],
                                  in_=segq[w % 4][:, :],
                                  func=AF.Copy).then_inc(segcp, 1)
                    sc.wait_ge(w1d, w + 1)
                    sc.activation(out=out1h_sb[:, w * W:(w + 1) * W],
                                  in_=out1_ps[w % 2][:, :], func=AF.Copy,
                                  accum_out=stats1_sb[:, w:w + 1])
                    sc.activation(out=sq_sb[w % 2][:HID, :W],
                                  in_=out1_ps[w % 2][:, :], func=AF.Square,
                                  accum_out=stats1_sb[:, NWC + w:NWC + w + 1]
                                  ).then_inc(dved1, 1)
                # sqrt for BN1
                sc.wait_ge(cfa, 1)
                sc.activation(out=stat_sb[:HID, 1:2], in_=stat_sb[:HID, 1:2],
                              func=AF.Sqrt).then_inc(cfb, 1)
                # BN1 apply per window (fused relu(a*x+b)) -> happ
                sc.wait_ge(cf1, 1)
                for w in range(NWC):
                    if w >= 2:
                        sc.wait_ge(hTd, w - 1)     # happ ring reuse
                    sc.activation(out=happ_sb[w % 2][:, :],
                                  in_=out1h_sb[:, w * W:(w + 1) * W],
                                  func=AF.Relu, bias=coef_sb[:HID, 0:1],
                                  scale=coef_sb[:HID, 1:2]).then_inc(hapA, 1)
                    # copy transpose result of previous window
                    sc.wait_ge(hTd, w + 1)
                    sc.activation(out=h_nm[:, w, :],
                                  in_=hT_ps[w % 2][:, :],
                                  func=AF.Copy).then_inc(hnm, 1)
                # L2 per-window: seg2f copy, out2 copy
                for w in range(NWC):
                    sc.wait_ge(pchunk2, wlast[w] + 1)
                    if w >= 2:
                        sc.wait_ge(w2d, w - 1)
                    sc.activation(out=s2f_sb[w % 2][:, :],
                                  in_=segq[w % 4][:HID, :],
                                  func=AF.Copy).then_inc(seg2cp, 1)
                    sc.wait_ge(w2d, w + 1)
                    sc.activation(out=out2_sb[:, w * EMB:(w + 1) * EMB],
                                  in_=out2_ps[w % 2][:, :],
                                  func=AF.Copy).then_inc(o2cp, 1)
                # L2 stats to sbuf, sqrt
                sc.wait_ge(stcnt, NWC)
                sc.activation(out=stat_sb[:EMB, 0:1], in_=st_ps[0][:EMB, :],
                              func=AF.Copy)
                sc.activation(out=stat_sb[:EMB, 1:2], in_=st_ps[1][:EMB, :],
                              func=AF.Copy).then_inc(st2c, 1)
                sc.wait_ge(cfa2, 1)
                sc.activation(out=stat_sb[:EMB, 1:2], in_=stat_sb[:EMB, 1:2],
                              func=AF.Sqrt).then_inc(cfb2, 1)
                # final pool copies
                sc.wait_ge(plm, NWC)
                for gh in range(GHALF):
                    a = sc.activation(out=pout_sb[:, gh * EMB:(gh + 1) * EMB],
                                      in_=pool_ps[gh][:, :], func=AF.Copy)
                    if gh == GHALF - 1:
                        a.then_inc(outc, 1)

            # ------------------------------------------------ TENSOR
            @block.tensor
            def _(pe):
                pe.wait_ge(io, 16 * NLOAD)
                # L1 chunk matmuls + per-window W1
                done_w1 = 0

                def drain_w1(upto):
                    nonlocal done_w1
                    while done_w1 < upto:
                        w = done_w1
                        pe.wait_ge(segcp, w + 1)
                        if w >= 1:
                            pe.wait_ge(dved1, w)   # out1_ps ring hazard
                        pe.matmul(out1_ps[w % 2][:, :], lhsT=w1_sb[:, :],
                                  rhs=segx_sb[w % 2][:, :], start=True,
                                  stop=True).then_inc(w1d, 1)
                        done_w1 += 1

                for t in range(n_chunks):
                    w = chunk_window[t]
                    sl = t // SLAB
                    pe.wait_ge(xls, 16 * (sl + 1))
                    pe.wait_ge(sdone, t + 1)
                    first, lastc = t == wfirst[w], t == wlast[w]
                    if first and w >= 4:
                        pe.wait_ge(segcp, w - 3)   # psum bank ring
                    pe.matmul(segq[w % 4][:, :],
                              lhsT=xs[sl % 2][:, t - sl * SLAB, :],
                              rhs=s1sl[t % 8][:, :],
                              start=first, stop=lastc).then_inc(pchunk, 1)
                    if lastc:
                        drain_w1(w)    # drain previous windows
                drain_w1(NWC)

                # BN1 transposes
                for w in range(NWC):
                    pe.wait_ge(hapA, w + 1)
                    if w >= 2:
                        pe.wait_ge(hnm, w - 1)
                    pe.transpose(out=hT_ps[w % 2][:, :],
                                 in_=happ_sb[w % 2][:, :],
                                 identity=id16_sb[:HID, :HID]).then_inc(
                                     hTd, 1)

                # L2 chunk matmuls + per-window W2 + stats
                done_w2 = 0
                done_st = 0

                def drain_l2(upto_w2, upto_st):
                    nonlocal done_w2, done_st
                    while done_w2 < upto_w2:
                        w = done_w2
                        pe.wait_ge(seg2cp, w + 1)
                        if w >= 1:
                            pe.wait_ge(o2cp, w)    # out2_ps ring hazard
                        pe.matmul(out2_ps[w % 2][:, :],
                                  lhsT=s2f_sb[w % 2][:, :],
                                  rhs=w2_sb[:, :], start=True,
                                  stop=True).then_inc(w2d, 1)
                        done_w2 += 1
                    while done_st < upto_st:
                        w = done_st
                        pe.wait_ge(sq2d, w + 1)
                        pe.matmul(st_ps[0][:EMB, :],
                                  lhsT=out2_sb[:, w * EMB:(w + 1) * EMB],
                                  rhs=msk_sb[:, w:w + 1],
                                  start=(w == 0), stop=False)
                        pe.matmul(st_ps[1][:EMB, :],
                                  lhsT=sq2_sb[w % 2][:, :],
                                  rhs=msk_sb[:, w:w + 1],
                                  start=False,
                                  stop=(w == NWC - 1)).then_inc(stcnt, 1)
                        done_st += 1

                uses = [0] * NBUF
                for ci, (cb, nchc) in enumerate(calls):
                    b = ci % NBUF
                    uses[b] += 1
                    pe.wait_ge(gs2[b], 16 * uses[b])
                    for k in range(nchc):
                        t = cb + k
                        w = chunk_window[t]
                        pe.wait_ge(s4done, t + 1)
                        first, lastc = t == wfirst[w], t == wlast[w]
                        if t == wfirst[w] and w >= 4:
                            pe.wait_ge(seg2cp, w - 3)
                        for q in range(4):
                            mm = pe.matmul(
                                segq[w % 4][:HID, :],
                                lhsT=mb2[b][:, k, q * HID:(q + 1) * HID],
                                rhs=s4sl[t % 8][:, q * 128:(q + 1) * 128],
                                start=(first and q == 0),
                                stop=(lastc and q == 3))
                            if q == 3:
                                mm.then_inc(pchunk2, 1)
                        if lastc:
                            drain_l2(w, max(0, w - 1))
                drain_l2(NWC, NWC)

                # pool matmuls
                for w in range(NWC):
                    pe.wait_ge(bn2r, w + 1)
                    pe.wait_ge(gG, w + 1)
                    for gh in range(GHALF):
                        mm = pe.matmul(
                            pool_ps[gh][:, :],
                            lhsT=gone_sb[w % 2][:, gh * 128:(gh + 1) * 128],
                            rhs=out2_sb[:, w * EMB:(w + 1) * EMB],
                            start=(w == 0 and gh == 0),
                            stop=(w == NWC - 1 and gh == GHALF - 1))
                        if gh == GHALF - 1:
                            mm.then_inc(plm, 1)

            # ------------------------------------------------ SYNC
            @block.sync
            def _(sy):
                for dst_ap, src_ap in (
                    (idx_sb[:, :], idx_d[:, :]),
                    (comb_sb[:, :], comb_d[:, :]),
                    (do_sb[:, :], do_d[:, :]),
                    (ew_sb[:, :], ew_d[:, :]),
                    (gid_sb[:, :], gid_d[:, :]),
                    (mskf_sb[:, :], msk_d[:, :]),
                    (w1_sb[:, :], w1_d[:, :]),
                    (w2_sb[:, :], w2_d[:, :]),
                    (bn_sb[:, :], bn_d[:, :]),
                ):
                    sy.dma_start(dst_ap, src_ap).then_inc(io, 16)
                # xg slab loads
                for sl in range(n_slabs):
                    if sl >= 2:
                        sy.wait_ge(pchunk, (sl - 1) * SLAB)
                    t0 = sl * SLAB
                    t1 = min(n_chunks, t0 + SLAB)
                    sy.dma_start(xs[sl % 2][:, :t1 - t0, :],
                                 xg_d[:, t0:t1, :]).then_inc(xls, 16)
                # BN1 stats -> AR1
                sy.wait_ge(stsr, 1)
                sy.dma_start(ar1_in[:, :], stat_sb[:HID, 0:2]).then_inc(
                    arS, 16)
                sy.wait_ge(cc, 1)
                sy.dma_start(stat_sb[:HID, 0:2], ar1_out[:, :]).then_inc(
                    ar1L, 16)
                # h table -> AG
                sy.wait_ge(hnm, NWC)
                sy.dma_start(
                    ag_in[:, :].rearrange("(t g) (m d) -> (g m) t d",
                                          g=32, m=4),
                    h_nm[:, :, :]).then_inc(agS, 16)
                # AR2
                sy.wait_ge(st2c, 1)
                sy.dma_start(ar2_in[:, :], stat_sb[:EMB, 0:2]).then_inc(
                    ar2S, 16)
                sy.wait_ge(cc, 3)
                sy.dma_start(stat_sb[:EMB, 0:2], ar2_out[:, :]).then_inc(
                    ar2L, 16)
                # bn2 coef rows -> replicated
                sy.wait_ge(cf2, 2)
                with nc.allow_non_contiguous_dma(reason="tiny 256-elem coef"):
                    sy.dma_start(bnrow[:, :].rearrange("c p -> p c"),
                                 coefh_sb[:EMB, 0:2]).then_inc(cfr, 16)
                sy.wait_ge(cfr, 16)
                rep = bass.AP(bnrow, 0, [[0, 128], [1, 2 * EMB]])
                sy.dma_start(coefr_sb[:, :], rep).then_inc(cfr, 16)
                # final output
                sy.wait_ge(outc, 1)
                sy.dma_start(
                    out_d[:, :, :].rearrange("g p d -> p g d"),
                    pout_sb[:, :].rearrange("p (g d) -> p g d", d=EMB),
                ).then_inc(ioh, 16)
                sy.wait_ge(ioh, 16)

    nc.compile()
    return nc


# ==================================================================== entry
def _make_in_maps(inputs, cfg, percore):
    HID, EMB = cfg["hid_dim"], cfg["emb_dim"]
    bnp = np.zeros((128, 6), np.float32)
    bnp[:HID, 0] = np.asarray(inputs["b1"], np.float32)
    bnp[:HID, 1] = np.asarray(inputs["g1"], np.float32)
    bnp[:HID, 2] = np.asarray(inputs["be1"], np.float32)
    bnp[:EMB, 3] = np.asarray(inputs["b2"], np.float32)
    bnp[:EMB, 4] = np.asarray(inputs["g2"], np.float32)
    bnp[:EMB, 5] = np.asarray(inputs["be2"], np.float32)
    w1 = np.asarray(inputs["W1"], np.float32).astype(np.float16)
    w2 = np.asarray(inputs["W2"], np.float32).astype(np.float16)
    return [dict(
        xg=percore["xg"][c], idx=percore["idx"][c], cmb=percore["comb"][c],
        dof=percore["dstoff"][c], ewt=percore["ew"][c],
        gid=percore["gid"][c], msk=percore["msk"][c], w1=w1, w2=w2, bnp=bnp,
    ) for c in range(cfg["n_cores"])]


def _run(inputs, cfg):
    x = np.asarray(inputs["x"], np.float32)
    layout, percore, slot = _host_prep(
        x, inputs["edge_index"], inputs["edge_weight"], inputs["batch_vec"],
        cfg)
    nc = _build(cfg, layout)

    NC = cfg["n_cores"]
    in_maps = _make_in_maps(inputs, cfg, percore)
    res = run_bass_kernel_spmd(nc, in_maps, list(range(NC)), trace=TRACE)

    NG, EMB = cfg["n_graphs"], cfg["emb_dim"]
    pool = np.zeros((NG, EMB), np.float64)
    for c in range(NC):
        p = res.results[c]["pool"].astype(np.float64)
        pool += p.reshape(NG, EMB)
    counts = np.bincount(np.asarray(inputs["batch_vec"], np.int64),
                         minlength=NG).astype(np.float64)
    pool /= np.maximum(counts, 1.0)[:, None]
    return pool.astype(np.float32), res


def kernel(**inputs):
    out, _ = _run(inputs, CFG_FULL)
    return out
